# revision 4
# baseline (speedup 1.0000x reference)
"""Causal self-attention (B=2, T=2048, D=2048, H=16, HD=128) on 8 TRN2 cores.

The per-iteration cost on this stack is dominated by host->device input
shipping (~1.1 ms per input tensor name + ~0.5 ms per core-MB), not by
on-device compute (~0.5 ms). So the kernel is organized to minimize I/O:

  - ONE packed bf16 input blob per core (~6.8 MB): x feature-row shard
    (256 rows of xT), this core's q/k/v head weights, a 256-column shard
    of W_o (rows permuted to match the AllGather layout), raw cos/sin.
  - On-device AllGather reassembles full xT from the 8 shards.
  - Tensor-parallel attention over heads (2 heads/core): QKV matmul,
    RMS norm, rotary, causal SDPA — all contractions on the partition
    dim, scores computed transposed, softmax denominator via all-ones
    matmul, causal mask via gpsimd affine_select (no mask input).
  - Per-head AllGather of the attention output yT, then a column-
    parallel output projection (each core computes 256 output features
    for all 4096 tokens) — needs only 1 MB of W_o per core instead of
    the full 8 MB.
  - bf16 output [256, 4096] (out-features x tokens), assembled and cast
    to f32 on the host.

Matmuls run in bf16 (fp32 is 4 cycles/row on the PE, bf16 is 1); PSUM
accumulation and softmax statistics stay fp32.
"""

import numpy as np

B, T, D = 2, 2048, 2048
H, HD = 16, 128
N_CORES = 8
HPC = H // N_CORES          # heads per core
NT = B * T                  # 4096 tokens, b-major
DC = D // 128               # 16 contraction chunks
NTT = NT // 512             # 8 token tiles
KT_PER_B = T // 128         # 16 k-tiles per batch row

# packed input blob regions (elements, bf16)
SZ_X = 256 * NT
SZ_WQK = 128 * DC * 512
SZ_WV = 128 * DC * 256
SZ_W3 = 128 * DC * 256
SZ_CS = 64 * T
OFF_X = 0
OFF_WQK = OFF_X + SZ_X
OFF_WV = OFF_WQK + SZ_WQK
OFF_W3 = OFF_WV + SZ_WV
OFF_COS = OFF_W3 + SZ_W3
OFF_SIN = OFF_COS + SZ_CS
BLOB = OFF_SIN + SZ_CS

_CACHE = {}


def _build(scale: float, reps: int = 1):
    import concourse.bacc as bacc
    import concourse.mybir as mybir
    import concourse.tile as tile

    f32 = mybir.dt.float32
    MM = mybir.dt.bfloat16
    EPS = float(np.finfo(np.float32).eps)

    nc = bacc.Bacc("TRN2", target_bir_lowering=False, debug=False,
                   num_devices=N_CORES)

    blob_d = nc.dram_tensor("blob", [BLOB], MM, kind="ExternalInput")
    y_d = nc.dram_tensor("y", [2 * 128, NT], MM, kind="ExternalOutput")

    Sq = mybir.ActivationFunctionType.Square
    Sqrt = mybir.ActivationFunctionType.Sqrt
    Exp = mybir.ActivationFunctionType.Exp
    Copy = mybir.ActivationFunctionType.Copy
    is_ge = mybir.AluOpType.is_ge
    bypass = mybir.AluOpType.bypass
    RG = [list(range(N_CORES))]

    def blob2d(off, p, f):
        return blob_d[off:off + p * f].rearrange("(p f) -> p f", f=f)

    with tile.TileContext(nc) as tc:
        with tc.tile_pool(name="dram", bufs=1, space="DRAM") as dram, \
             tc.tile_pool(name="res", bufs=1) as res:
            # residents: rotated q/k (m-chunks q0,q1,k0,k1), v in
            # [token, hd] layout, cos/sin, all-ones, o-proj weights
            qk_sb = res.tile([128, 4 * NT], MM, tag="qk")
            v_sb = res.tile([128, (NT // 128) * (HPC * HD)], MM, tag="v")
            cs_sb = res.tile([128, 2 * T], MM, tag="cs")
            w3_sb = res.tile([128, DC * 256], MM, tag="w3")
            ones_sb = res.tile([128, 128], MM, tag="ones")
            eps_sb = res.tile([128, 1], f32, tag="eps")
            nc.vector.memset(eps_sb[:], EPS)
            nc.vector.memset(ones_sb[:], 1.0)

            for _rep in range(reps):
                # Shared collective outputs allow a single writer inst, so
                # allocate fresh AG tiles per rep (reps>1 is diagnostics-only)
                agx_in = dram.tile([256, NT], MM, tag="agx_in",
                                   name=f"agx_in_r{_rep}")
                agx_out = dram.tile([D, NT], MM, tag="agx_out",
                                    name=f"agx_out_r{_rep}", addr_space="Shared")
                agy_in = [dram.tile([128, NT], MM, tag=f"agy_in{h}",
                                    name=f"agy_in{h}_r{_rep}") for h in range(HPC)]
                agy_out = [dram.tile([1024, NT], MM, tag=f"agy_out{h}",
                                     name=f"agy_out{h}_r{_rep}",
                                     addr_space="Shared") for h in range(HPC)]
                # x AllGather first: everything in phase 1 waits on it
                for hf in range(2):
                    nc.sync.dma_start(
                        out=agx_in[hf * 128:(hf + 1) * 128, :],
                        in_=blob2d(OFF_X + hf * 128 * NT, 128, NT))
                nc.gpsimd.collective_compute(
                    "AllGather", bypass, replica_groups=RG,
                    ins=[agx_in.opt()], outs=[agx_out.opt()])

                # cos/sin: C = [cosT; cosT], S = [-sinT; sinT]
                nc.sync.dma_start(out=cs_sb[0:64, 0:T],
                                  in_=blob2d(OFF_COS, 64, T))
                nc.sync.dma_start(out=cs_sb[64:128, 0:T],
                                  in_=blob2d(OFF_COS, 64, T))
                nc.sync.dma_start(out=cs_sb[64:128, T:2 * T],
                                  in_=blob2d(OFF_SIN, 64, T))
                nc.sync.dma_start(out=cs_sb[0:64, T:2 * T],
                                  in_=blob2d(OFF_SIN, 64, T))
                nc.scalar.activation(cs_sb[0:64, T:2 * T],
                                     cs_sb[0:64, T:2 * T], Copy, scale=-1.0)
                nc.sync.dma_start(out=w3_sb[:],
                                  in_=blob2d(OFF_W3, 128, DC * 256))

                # ---------------- Phase 1: QKV + RMS norm + rotary ----------------
                with tc.tile_pool(name="p1", bufs=1) as p1, \
                     tc.tile_pool(name="xs", bufs=3) as xs, \
                     tc.tile_pool(name="st", bufs=3) as st, \
                     tc.tile_pool(name="ps1", bufs=2, space="PSUM") as ps1:
                    wqk_sb = p1.tile([128, DC * 512], MM, tag="wqk")
                    wv_sb = p1.tile([128, DC * 256], MM, tag="wv")
                    nc.sync.dma_start(out=wv_sb[:],
                                      in_=blob2d(OFF_WV, 128, DC * 256))
                    nc.sync.dma_start(out=wqk_sb[:],
                                      in_=blob2d(OFF_WQK, 128, DC * 512))

                    for n in range(NTT):
                        xblk = xs.tile([128, DC * 512], MM, tag="xblk")
                        for cg in range(4):
                            nc.sync.dma_start(
                                out=xblk[:, cg * 4 * 512:(cg + 1) * 4 * 512]
                                    .rearrange("p (c f) -> p c f", f=512),
                                in_=agx_out[cg * 512:(cg + 1) * 512,
                                            n * 512:(n + 1) * 512]
                                    .rearrange("(c p) f -> p c f", p=128))
                        # v projection: [token, hd] layout
                        for c4 in range(4):
                            vps = ps1.tile([128, HPC * HD], f32, tag="vps")
                            for dc in range(DC):
                                nc.tensor.matmul(
                                    vps[:],
                                    xblk[:, dc * 512 + c4 * 128: dc * 512 + (c4 + 1) * 128],
                                    wv_sb[:, dc * 256:(dc + 1) * 256],
                                    start=(dc == 0), stop=(dc == DC - 1))
                            tcg = n * 4 + c4
                            nc.vector.tensor_copy(v_sb[:, tcg * 256:(tcg + 1) * 256], vps[:])
                        # q/k projection + rms + rotary, m-chunks q0,q1,k0,k1
                        for m in range(4):
                            qps = ps1.tile([128, 512], f32, tag="qps")
                            for dc in range(DC):
                                nc.tensor.matmul(
                                    qps[:],
                                    wqk_sb[:, dc * 512 + m * 128: dc * 512 + (m + 1) * 128],
                                    xblk[:, dc * 512:(dc + 1) * 512],
                                    start=(dc == 0), stop=(dc == DC - 1))
                            sq = st.tile([128, 512], MM, tag="sq")
                            nc.scalar.activation(sq[:], qps[:], Sq)
                            ssq = ps1.tile([128, 512], f32, tag="ssq")
                            nc.tensor.matmul(ssq[:], ones_sb[:], sq[:], start=True, stop=True)
                            rms = st.tile([128, 512], f32, tag="rms")
                            nc.scalar.activation(rms[:], ssq[:], Sqrt, bias=eps_sb[:], scale=1.0 / HD)
                            r = st.tile([128, 512], f32, tag="r")
                            nc.vector.reciprocal(r[:], rms[:])
                            qn = st.tile([128, 512], MM, tag="qn")
                            nc.vector.tensor_mul(qn[:], qps[:], r[:])
                            # rotary: y = qn*C + swap(qn)*S  with S = [-sin; sin]
                            tsw = st.tile([128, 512], MM, tag="tsw")
                            tb = (n % 4) * 512
                            ctile = cs_sb[:, tb:tb + 512]
                            stile = cs_sb[:, T + tb:T + tb + 512]
                            nc.vector.tensor_mul(tsw[0:64, :], qn[64:128, :], stile[64:128, :])
                            nc.vector.tensor_mul(tsw[64:128, :], qn[0:64, :], stile[0:64, :])
                            dst = qk_sb[:, m * NT + n * 512: m * NT + (n + 1) * 512]
                            nc.vector.tensor_mul(dst, qn[:], ctile)
                            nc.vector.tensor_add(dst, dst, tsw[:])

                # ---------------- Phase 2: attention + per-head AllGather ----------------
                with tc.tile_pool(name="p2", bufs=4) as p2, \
                     tc.tile_pool(name="p2b", bufs=2) as p2b, \
                     tc.tile_pool(name="pss", bufs=2, space="PSUM") as pss, \
                     tc.tile_pool(name="psd", bufs=2, space="PSUM") as psd, \
                     tc.tile_pool(name="psy", bufs=2, space="PSUM") as psy:
                    for h in range(HPC):
                        qoff = h * NT
                        koff = (2 + h) * NT
                        for b in range(B):
                            for qj in range(4):
                                yps = psy.tile([128, 512], f32, tag="yps")
                                dps = psd.tile([128, 512], f32, tag="dps")
                                nkt = 4 * qj + 4
                                qbase = qoff + b * T + qj * 512
                                for kb in range(nkt):
                                    # diagonal blocks: only q-columns >= 128*m live
                                    lo = max(0, (kb - 4 * qj) * 128)
                                    sps = pss.tile([128, 512], f32, tag="sps")
                                    nc.tensor.matmul(
                                        sps[:, lo:],
                                        qk_sb[:, koff + b * T + kb * 128: koff + b * T + (kb + 1) * 128],
                                        qk_sb[:, qbase + lo: qbase + 512],
                                        start=True, stop=True)
                                    e = p2.tile([128, 512], MM, tag="e")
                                    nc.scalar.activation(e[:, lo:], sps[:, lo:], Exp, scale=scale)
                                    if kb >= 4 * qj:
                                        # causal: keep col j (>=lo) iff local
                                        # k-row p <= j - lo  (j - p - lo >= 0)
                                        nc.gpsimd.affine_select(
                                            out=e[:, lo:], in_=e[:, lo:],
                                            pattern=[[1, 512 - lo]],
                                            compare_op=is_ge, fill=0.0,
                                            base=0, channel_multiplier=-1)
                                    nc.tensor.matmul(dps[:, lo:], ones_sb[:], e[:, lo:],
                                                     start=(kb == 0), stop=(kb == nkt - 1))
                                    tcg = b * KT_PER_B + kb
                                    nc.tensor.matmul(
                                        yps[:, lo:],
                                        v_sb[:, tcg * 256 + h * 128: tcg * 256 + (h + 1) * 128],
                                        e[:, lo:],
                                        start=(kb == 0), stop=(kb == nkt - 1))
                                rcp = p2b.tile([128, 512], f32, tag="rcp")
                                nc.vector.reciprocal(rcp[:], dps[:])
                                yn = p2b.tile([128, 512], MM, tag="yn")
                                nc.vector.tensor_mul(yn[:], yps[:], rcp[:])
                                s = b * 4 + qj
                                nc.sync.dma_start(
                                    out=agy_in[h][:, s * 512:(s + 1) * 512],
                                    in_=yn[:])
                        # head h's AllGather overlaps head h+1's attention
                        nc.gpsimd.collective_compute(
                            "AllGather", bypass, replica_groups=RG,
                            ins=[agy_in[h].opt()], outs=[agy_out[h].opt()])

                # ---------------- Phase 3: column-parallel o-proj ----------------
                # ytile d-chunk dc: rows of agy_out[dc//8] block (dc%8);
                # w3 host layout is permuted to match.
                with tc.tile_pool(name="yt", bufs=2) as ytp, \
                     tc.tile_pool(name="ob", bufs=2) as obp, \
                     tc.tile_pool(name="ps3", bufs=2, space="PSUM") as ps3:
                    for tt in range(NTT):
                        ytile = ytp.tile([128, DC * 512], MM, tag="ytile")
                        for h in range(HPC):
                            nc.sync.dma_start(
                                out=ytile[:, h * 8 * 512:(h + 1) * 8 * 512]
                                    .rearrange("p (c f) -> p c f", f=512),
                                in_=agy_out[h][0:1024, tt * 512:(tt + 1) * 512]
                                    .rearrange("(c p) f -> p c f", p=128))
                        for oc2 in range(2):
                            ps = ps3.tile([128, 512], f32, tag="ops")
                            for dc in range(DC):
                                nc.tensor.matmul(
                                    ps[:],
                                    w3_sb[:, dc * 256 + oc2 * 128: dc * 256 + (oc2 + 1) * 128],
                                    ytile[:, dc * 512:(dc + 1) * 512],
                                    start=(dc == 0), stop=(dc == DC - 1))
                            ob = obp.tile([128, 512], MM, tag="ob")
                            nc.scalar.activation(ob[:], ps[:], Copy)
                            nc.sync.dma_start(
                                out=y_d[oc2 * 128:(oc2 + 1) * 128,
                                        tt * 512:(tt + 1) * 512],
                                in_=ob[:])

    nc.compile()
    return nc


def _prep_inputs(x, W, cos, sin):
    import concourse.mybir as mybir
    bf = mybir.dt.np(mybir.dt.bfloat16)

    xT = np.ascontiguousarray(x.reshape(NT, D).T).astype(bf)  # [D, NT]
    cosT = cos.T.astype(bf)  # [64, T]
    sinT = sin.T.astype(bf)
    W3T = W[3].T  # [d_in, d_out]
    # AllGather row order: AG#h stacks head (2c+h) of core c at block c
    blocks = [2 * c for c in range(N_CORES)] + [2 * c + 1 for c in range(N_CORES)]
    rows = np.concatenate([np.arange(b * 128, (b + 1) * 128) for b in blocks])
    W3p = W3T[rows]  # [d_in permuted, d_out]

    in_maps = []
    for c in range(N_CORES):
        r0 = c * HPC * HD
        wqk = np.concatenate([W[0][r0:r0 + 256], W[1][r0:r0 + 256]], 0).T  # [D, 512]
        wqk_sb = wqk.reshape(DC, 128, 512).transpose(1, 0, 2)
        wv = W[2][r0:r0 + 256].T  # [D, 256]
        wv_sb = wv.reshape(DC, 128, 256).transpose(1, 0, 2)
        w3_sb = W3p[:, r0:r0 + 256].reshape(DC, 128, 256).transpose(1, 0, 2)

        blob = np.empty(BLOB, bf)
        blob[OFF_X:OFF_X + SZ_X] = xT[c * 256:(c + 1) * 256].reshape(-1)
        blob[OFF_WQK:OFF_WQK + SZ_WQK] = wqk_sb.astype(bf).reshape(128, -1).reshape(-1)
        blob[OFF_WV:OFF_WV + SZ_WV] = wv_sb.astype(bf).reshape(128, -1).reshape(-1)
        blob[OFF_W3:OFF_W3 + SZ_W3] = w3_sb.astype(bf).reshape(128, -1).reshape(-1)
        blob[OFF_COS:OFF_COS + SZ_CS] = cosT.reshape(-1)
        blob[OFF_SIN:OFF_SIN + SZ_CS] = sinT.reshape(-1)
        in_maps.append({"blob": blob})
    return in_maps


def kernel(x, W, cos, sin, scale):
    from concourse.bass_utils import run_bass_kernel_spmd

    x = np.asarray(x, dtype=np.float32)
    W = np.asarray(W, dtype=np.float32)
    cos = np.asarray(cos, dtype=np.float32)
    sin = np.asarray(sin, dtype=np.float32)
    sc = float(np.asarray(scale))

    if sc not in _CACHE:
        _CACHE[sc] = _build(sc)
    nc = _CACHE[sc]

    in_maps = _prep_inputs(x, W, cos, sin)
    out = run_bass_kernel_spmd(nc, in_maps, core_ids=list(range(N_CORES)))
    yT = np.concatenate(
        [np.asarray(out.results[c]["y"], dtype=np.float32) for c in range(N_CORES)],
        axis=0)  # [D, NT]
    return np.ascontiguousarray(yT.T).reshape(B, T, D)


# revision 10
# speedup vs baseline: 1.0422x; 1.0422x over previous
"""Causal self-attention (B=2, T=2048, D=2048, H=16, HD=128) on 8 TRN2 cores.

The per-iteration cost on this stack is dominated by host->device input
shipping (~1.1 ms per input tensor name + ~0.5 ms per core-MB), not by
on-device compute (~0.5 ms). So the kernel is organized to minimize I/O:

  - ONE packed bf16 input blob per core (~6.8 MB): x feature-row shard
    (256 rows of xT), this core's q/k/v head weights, a 256-column shard
    of W_o (rows permuted to match the AllGather layout), raw cos/sin.
  - On-device AllGather reassembles full xT from the 8 shards.
  - Tensor-parallel attention over heads (2 heads/core): QKV matmul,
    RMS norm, rotary, causal SDPA — all contractions on the partition
    dim, scores computed transposed, softmax denominator via all-ones
    matmul, causal mask via gpsimd affine_select (no mask input).
  - Per-head AllGather of the attention output yT, then a column-
    parallel output projection (each core computes 256 output features
    for all 4096 tokens) — needs only 1 MB of W_o per core instead of
    the full 8 MB.
  - bf16 output [256, 4096] (out-features x tokens), assembled and cast
    to f32 on the host.

Matmuls run in bf16 (fp32 is 4 cycles/row on the PE, bf16 is 1); PSUM
accumulation and softmax statistics stay fp32.
"""

import numpy as np

B, T, D = 2, 2048, 2048
H, HD = 16, 128
N_CORES = 8
HPC = H // N_CORES          # heads per core
NT = B * T                  # 4096 tokens, b-major
DC = D // 128               # 16 contraction chunks
NTT = NT // 512             # 8 token tiles
KT_PER_B = T // 128         # 16 k-tiles per batch row

# packed input blob regions (elements, bf16)
SZ_X = 256 * NT
SZ_WQK = 128 * DC * 512
SZ_WV = 128 * DC * 256
SZ_W3 = 128 * DC * 256
SZ_CS = 64 * T
OFF_X = 0
OFF_WQK = OFF_X + SZ_X
OFF_WV = OFF_WQK + SZ_WQK
OFF_W3 = OFF_WV + SZ_WV
OFF_COS = OFF_W3 + SZ_W3
OFF_SIN = OFF_COS + SZ_CS
BLOB = OFF_SIN + SZ_CS

_CACHE = {}


def _build(scale: float, reps: int = 1):
    import concourse.bacc as bacc
    import concourse.mybir as mybir
    import concourse.tile as tile

    f32 = mybir.dt.float32
    MM = mybir.dt.bfloat16
    EPS = float(np.finfo(np.float32).eps)

    nc = bacc.Bacc("TRN2", target_bir_lowering=False, debug=False,
                   num_devices=N_CORES)

    blob_d = nc.dram_tensor("blob", [BLOB], MM, kind="ExternalInput")
    y_d = nc.dram_tensor("y", [2 * 128, NT], MM, kind="ExternalOutput")

    Sq = mybir.ActivationFunctionType.Square
    Sqrt = mybir.ActivationFunctionType.Sqrt
    Exp = mybir.ActivationFunctionType.Exp
    Copy = mybir.ActivationFunctionType.Copy
    is_ge = mybir.AluOpType.is_ge
    bypass = mybir.AluOpType.bypass
    RG = [list(range(N_CORES))]

    def blob2d(off, p, f):
        return blob_d[off:off + p * f].rearrange("(p f) -> p f", f=f)

    with tile.TileContext(nc) as tc:
        with tc.tile_pool(name="dram", bufs=1, space="DRAM") as dram, \
             tc.tile_pool(name="res", bufs=1) as res:
            # residents: rotated q/k (m-chunks q0,q1,k0,k1), v in
            # [token, hd] layout, cos/sin, all-ones, o-proj weights
            qk_sb = res.tile([128, 4 * NT], MM, tag="qk")
            v_sb = res.tile([128, (NT // 128) * (HPC * HD)], MM, tag="v")
            cs_sb = res.tile([128, 2 * T], MM, tag="cs")
            w3_sb = res.tile([128, DC * 256], MM, tag="w3")
            ones_sb = res.tile([128, 128], MM, tag="ones")
            eps_sb = res.tile([128, 1], f32, tag="eps")
            nc.vector.memset(eps_sb[:], EPS)
            nc.vector.memset(ones_sb[:], 1.0)

            for _rep in range(reps):
                # Shared collective outputs allow a single writer inst, so
                # allocate fresh AG tiles per rep (reps>1 is diagnostics-only)
                agx_in = [dram.tile([256, 1024], MM, tag=f"agx_in{g}",
                                    name=f"agx_in{g}_r{_rep}") for g in range(4)]
                agx_out = [dram.tile([D, 1024], MM, tag=f"agx_out{g}",
                                     name=f"agx_out{g}_r{_rep}",
                                     addr_space="Shared") for g in range(4)]
                agy_in = [dram.tile([128, NT], MM, tag=f"agy_in{h}",
                                    name=f"agy_in{h}_r{_rep}") for h in range(HPC)]
                agy_out = [dram.tile([1024, NT], MM, tag=f"agy_out{h}",
                                     name=f"agy_out{h}_r{_rep}",
                                     addr_space="Shared") for h in range(HPC)]
                # x AllGather, chunked by 1024-token column groups so phase 1
                # can start after the first chunk lands
                for g in range(4):
                    for hf in range(2):
                        nc.sync.dma_start(
                            out=agx_in[g][hf * 128:(hf + 1) * 128, :],
                            in_=blob2d(OFF_X + g * 256 * 1024 + hf * 128 * 1024,
                                       128, 1024))
                    nc.gpsimd.collective_compute(
                        "AllGather", bypass, replica_groups=RG,
                        ins=[agx_in[g].opt()], outs=[agx_out[g].opt()])

                # cos/sin: C = [cosT; cosT], S = [-sinT; sinT]
                nc.sync.dma_start(out=cs_sb[0:64, 0:T],
                                  in_=blob2d(OFF_COS, 64, T))
                nc.sync.dma_start(out=cs_sb[64:128, 0:T],
                                  in_=blob2d(OFF_COS, 64, T))
                nc.sync.dma_start(out=cs_sb[64:128, T:2 * T],
                                  in_=blob2d(OFF_SIN, 64, T))
                nc.sync.dma_start(out=cs_sb[0:64, T:2 * T],
                                  in_=blob2d(OFF_SIN, 64, T))
                nc.scalar.activation(cs_sb[0:64, T:2 * T],
                                     cs_sb[0:64, T:2 * T], Copy, scale=-1.0)
                nc.sync.dma_start(out=w3_sb[:],
                                  in_=blob2d(OFF_W3, 128, DC * 256))

                # ---------------- Phase 1: QKV + RMS norm + rotary ----------------
                with tc.tile_pool(name="p1", bufs=1) as p1, \
                     tc.tile_pool(name="xs", bufs=3) as xs, \
                     tc.tile_pool(name="st", bufs=3) as st, \
                     tc.tile_pool(name="ps1", bufs=2, space="PSUM") as ps1:
                    wqk_sb = p1.tile([128, DC * 512], MM, tag="wqk")
                    wv_sb = p1.tile([128, DC * 256], MM, tag="wv")
                    nc.sync.dma_start(out=wv_sb[:],
                                      in_=blob2d(OFF_WV, 128, DC * 256))
                    nc.sync.dma_start(out=wqk_sb[:],
                                      in_=blob2d(OFF_WQK, 128, DC * 512))

                    for n in range(NTT):
                        xblk = xs.tile([128, DC * 512], MM, tag="xblk")
                        nh = (n % 2) * 512
                        for cg in range(4):
                            nc.sync.dma_start(
                                out=xblk[:, cg * 4 * 512:(cg + 1) * 4 * 512]
                                    .rearrange("p (c f) -> p c f", f=512),
                                in_=agx_out[n // 2][cg * 512:(cg + 1) * 512,
                                                    nh:nh + 512]
                                    .rearrange("(c p) f -> p c f", p=128))
                        # v projection: [token, hd] layout
                        for c4 in range(4):
                            vps = ps1.tile([128, HPC * HD], f32, tag="vps")
                            for dc in range(DC):
                                nc.tensor.matmul(
                                    vps[:],
                                    xblk[:, dc * 512 + c4 * 128: dc * 512 + (c4 + 1) * 128],
                                    wv_sb[:, dc * 256:(dc + 1) * 256],
                                    start=(dc == 0), stop=(dc == DC - 1))
                            tcg = n * 4 + c4
                            nc.vector.tensor_copy(v_sb[:, tcg * 256:(tcg + 1) * 256], vps[:])
                        # q/k projection + rms + rotary, m-chunks q0,q1,k0,k1
                        for m in range(4):
                            qps = ps1.tile([128, 512], f32, tag="qps")
                            for dc in range(DC):
                                nc.tensor.matmul(
                                    qps[:],
                                    wqk_sb[:, dc * 512 + m * 128: dc * 512 + (m + 1) * 128],
                                    xblk[:, dc * 512:(dc + 1) * 512],
                                    start=(dc == 0), stop=(dc == DC - 1))
                            sq = st.tile([128, 512], MM, tag="sq")
                            nc.scalar.activation(sq[:], qps[:], Sq)
                            ssq = ps1.tile([128, 512], f32, tag="ssq")
                            nc.tensor.matmul(ssq[:], ones_sb[:], sq[:], start=True, stop=True)
                            rms = st.tile([128, 512], f32, tag="rms")
                            nc.scalar.activation(rms[:], ssq[:], Sqrt, bias=eps_sb[:], scale=1.0 / HD)
                            r = st.tile([128, 512], f32, tag="r")
                            nc.vector.reciprocal(r[:], rms[:])
                            qn = st.tile([128, 512], MM, tag="qn")
                            nc.vector.tensor_mul(qn[:], qps[:], r[:])
                            # rotary: y = qn*C + swap(qn)*S  with S = [-sin; sin]
                            tsw = st.tile([128, 512], MM, tag="tsw")
                            tb = (n % 4) * 512
                            ctile = cs_sb[:, tb:tb + 512]
                            stile = cs_sb[:, T + tb:T + tb + 512]
                            nc.vector.tensor_mul(tsw[0:64, :], qn[64:128, :], stile[64:128, :])
                            nc.vector.tensor_mul(tsw[64:128, :], qn[0:64, :], stile[0:64, :])
                            dst = qk_sb[:, m * NT + n * 512: m * NT + (n + 1) * 512]
                            nc.vector.tensor_mul(dst, qn[:], ctile)
                            nc.vector.tensor_add(dst, dst, tsw[:])

                # ---------------- Phase 2: attention + per-head AllGather ----------------
                with tc.tile_pool(name="p2", bufs=6) as p2, \
                     tc.tile_pool(name="p2b", bufs=2) as p2b, \
                     tc.tile_pool(name="pss", bufs=3, space="PSUM") as pss, \
                     tc.tile_pool(name="psd", bufs=2, space="PSUM") as psd, \
                     tc.tile_pool(name="psy", bufs=2, space="PSUM") as psy:
                    for h in range(HPC):
                        qoff = h * NT
                        koff = (2 + h) * NT
                        for b in range(B):
                            for qj in range(4):
                                yps = psy.tile([128, 512], f32, tag="yps")
                                dps = psd.tile([128, 512], f32, tag="dps")
                                nkt = 4 * qj + 4
                                qbase = qoff + b * T + qj * 512
                                for kb in range(nkt):
                                    # diagonal blocks: only q-columns >= 128*m live
                                    lo = max(0, (kb - 4 * qj) * 128)
                                    sps = pss.tile([128, 512], f32, tag="sps")
                                    nc.tensor.matmul(
                                        sps[:, lo:],
                                        qk_sb[:, koff + b * T + kb * 128: koff + b * T + (kb + 1) * 128],
                                        qk_sb[:, qbase + lo: qbase + 512],
                                        start=True, stop=True)
                                    e = p2.tile([128, 512], MM, tag="e")
                                    nc.scalar.activation(e[:, lo:], sps[:, lo:], Exp, scale=scale)
                                    if kb >= 4 * qj:
                                        # causal: keep col j (>=lo) iff local
                                        # k-row p <= j - lo  (j - p - lo >= 0)
                                        nc.gpsimd.affine_select(
                                            out=e[:, lo:], in_=e[:, lo:],
                                            pattern=[[1, 512 - lo]],
                                            compare_op=is_ge, fill=0.0,
                                            base=0, channel_multiplier=-1)
                                    nc.tensor.matmul(dps[:, lo:], ones_sb[:], e[:, lo:],
                                                     start=(kb == 0), stop=(kb == nkt - 1))
                                    tcg = b * KT_PER_B + kb
                                    nc.tensor.matmul(
                                        yps[:, lo:],
                                        v_sb[:, tcg * 256 + h * 128: tcg * 256 + (h + 1) * 128],
                                        e[:, lo:],
                                        start=(kb == 0), stop=(kb == nkt - 1))
                                rcp = p2b.tile([128, 512], f32, tag="rcp")
                                nc.vector.reciprocal(rcp[:], dps[:])
                                yn = p2b.tile([128, 512], MM, tag="yn")
                                nc.vector.tensor_mul(yn[:], yps[:], rcp[:])
                                s = b * 4 + qj
                                nc.sync.dma_start(
                                    out=agy_in[h][:, s * 512:(s + 1) * 512],
                                    in_=yn[:])
                        # head h's AllGather overlaps head h+1's attention
                        nc.gpsimd.collective_compute(
                            "AllGather", bypass, replica_groups=RG,
                            ins=[agy_in[h].opt()], outs=[agy_out[h].opt()])

                # ---------------- Phase 3: column-parallel o-proj ----------------
                # ytile d-chunk dc: rows of agy_out[dc//8] block (dc%8);
                # w3 host layout is permuted to match. Two-pass accumulation:
                # dc 0-7 (head-0 data, available after AG#0) for 8 chains
                # first, so the tensor engine has work while AG#1 lands.
                with tc.tile_pool(name="yt", bufs=4) as ytp, \
                     tc.tile_pool(name="ob", bufs=4) as obp, \
                     tc.tile_pool(name="ps3", bufs=8, space="PSUM") as ps3:
                    for grp in range(2):
                        yts = []
                        for tl in range(4):
                            tt = grp * 4 + tl
                            ytile = ytp.tile([128, DC * 512], MM, tag="ytile")
                            for h in range(HPC):
                                nc.sync.dma_start(
                                    out=ytile[:, h * 8 * 512:(h + 1) * 8 * 512]
                                        .rearrange("p (c f) -> p c f", f=512),
                                    in_=agy_out[h][0:1024, tt * 512:(tt + 1) * 512]
                                        .rearrange("(c p) f -> p c f", p=128))
                            yts.append(ytile)
                        pst = [ps3.tile([128, 512], f32, tag="ops",
                                        name=f"ops{grp}_{i}")
                               for i in range(8)]
                        for half in range(2):
                            for tl in range(4):
                                for oc2 in range(2):
                                    ps = pst[tl * 2 + oc2]
                                    for dc in range(half * 8, half * 8 + 8):
                                        nc.tensor.matmul(
                                            ps[:],
                                            w3_sb[:, dc * 256 + oc2 * 128: dc * 256 + (oc2 + 1) * 128],
                                            yts[tl][:, dc * 512:(dc + 1) * 512],
                                            start=(dc == 0), stop=(dc == DC - 1))
                        for tl in range(4):
                            tt = grp * 4 + tl
                            for oc2 in range(2):
                                ob = obp.tile([128, 512], MM, tag="ob")
                                nc.scalar.activation(ob[:], pst[tl * 2 + oc2][:], Copy)
                                nc.sync.dma_start(
                                    out=y_d[oc2 * 128:(oc2 + 1) * 128,
                                            tt * 512:(tt + 1) * 512],
                                    in_=ob[:])

    nc.compile()
    return nc


def _prep_inputs(x, W, cos, sin):
    import concourse.mybir as mybir
    bf = mybir.dt.np(mybir.dt.bfloat16)

    xT = np.ascontiguousarray(x.reshape(NT, D).T).astype(bf)  # [D, NT]
    cosT = cos.T.astype(bf)  # [64, T]
    sinT = sin.T.astype(bf)
    W3T = W[3].T  # [d_in, d_out]
    # AllGather row order: AG#h stacks head (2c+h) of core c at block c
    blocks = [2 * c for c in range(N_CORES)] + [2 * c + 1 for c in range(N_CORES)]
    rows = np.concatenate([np.arange(b * 128, (b + 1) * 128) for b in blocks])
    W3p = W3T[rows]  # [d_in permuted, d_out]

    in_maps = []
    for c in range(N_CORES):
        r0 = c * HPC * HD
        wqk = np.concatenate([W[0][r0:r0 + 256], W[1][r0:r0 + 256]], 0).T  # [D, 512]
        wqk_sb = wqk.reshape(DC, 128, 512).transpose(1, 0, 2)
        wv = W[2][r0:r0 + 256].T  # [D, 256]
        wv_sb = wv.reshape(DC, 128, 256).transpose(1, 0, 2)
        w3_sb = W3p[:, r0:r0 + 256].reshape(DC, 128, 256).transpose(1, 0, 2)

        blob = np.empty(BLOB, bf)
        xs = xT[c * 256:(c + 1) * 256]
        for g in range(4):
            blob[OFF_X + g * 256 * 1024:OFF_X + (g + 1) * 256 * 1024] = \
                np.ascontiguousarray(xs[:, g * 1024:(g + 1) * 1024]).reshape(-1)
        blob[OFF_WQK:OFF_WQK + SZ_WQK] = wqk_sb.astype(bf).reshape(128, -1).reshape(-1)
        blob[OFF_WV:OFF_WV + SZ_WV] = wv_sb.astype(bf).reshape(128, -1).reshape(-1)
        blob[OFF_W3:OFF_W3 + SZ_W3] = w3_sb.astype(bf).reshape(128, -1).reshape(-1)
        blob[OFF_COS:OFF_COS + SZ_CS] = cosT.reshape(-1)
        blob[OFF_SIN:OFF_SIN + SZ_CS] = sinT.reshape(-1)
        in_maps.append({"blob": blob})
    return in_maps


def kernel(x, W, cos, sin, scale):
    from concourse.bass_utils import run_bass_kernel_spmd

    x = np.asarray(x, dtype=np.float32)
    W = np.asarray(W, dtype=np.float32)
    cos = np.asarray(cos, dtype=np.float32)
    sin = np.asarray(sin, dtype=np.float32)
    sc = float(np.asarray(scale))

    if sc not in _CACHE:
        _CACHE[sc] = _build(sc)
    nc = _CACHE[sc]

    in_maps = _prep_inputs(x, W, cos, sin)
    out = run_bass_kernel_spmd(nc, in_maps, core_ids=list(range(N_CORES)))
    yT = np.concatenate(
        [np.asarray(out.results[c]["y"], dtype=np.float32) for c in range(N_CORES)],
        axis=0)  # [D, NT]
    return np.ascontiguousarray(yT.T).reshape(B, T, D)


# revision 13
# speedup vs baseline: 1.0917x; 1.0475x over previous
"""Causal self-attention (B=2, T=2048, D=2048, H=16, HD=128) on 8 TRN2 cores.

The per-iteration cost on this stack is dominated by host->device input
shipping (~1.1 ms per input tensor name + ~0.5 ms per core-MB), not by
on-device compute (~0.5 ms). So the kernel is organized to minimize I/O:

  - ONE packed bf16 input blob per core (~6.8 MB): x feature-row shard
    (256 rows of xT), this core's q/k/v head weights, a 256-column shard
    of W_o (rows permuted to match the AllGather layout), raw cos/sin.
  - On-device AllGather reassembles full xT from the 8 shards.
  - Tensor-parallel attention over heads (2 heads/core): QKV matmul,
    RMS norm, rotary, causal SDPA — all contractions on the partition
    dim, scores computed transposed, softmax denominator via all-ones
    matmul, causal mask via gpsimd affine_select (no mask input).
  - Per-head AllGather of the attention output yT, then a column-
    parallel output projection (each core computes 256 output features
    for all 4096 tokens) — needs only 1 MB of W_o per core instead of
    the full 8 MB.
  - bf16 output [256, 4096] (out-features x tokens), assembled and cast
    to f32 on the host.

Matmuls run in bf16 (fp32 is 4 cycles/row on the PE, bf16 is 1); PSUM
accumulation and softmax statistics stay fp32.
"""

import numpy as np

B, T, D = 2, 2048, 2048
H, HD = 16, 128
N_CORES = 8
HPC = H // N_CORES          # heads per core
NT = B * T                  # 4096 tokens, b-major
DC = D // 128               # 16 contraction chunks
NTT = NT // 512             # 8 token tiles
KT_PER_B = T // 128         # 16 k-tiles per batch row

# packed input blob regions (elements, bf16)
SZ_X = 256 * NT
SZ_WQK = 128 * DC * 512
SZ_WV = 128 * DC * 256
SZ_W3 = 128 * DC * 256
SZ_CS = 64 * T
OFF_X = 0
OFF_WQK = OFF_X + SZ_X
OFF_WV = OFF_WQK + SZ_WQK
OFF_W3 = OFF_WV + SZ_WV
OFF_COS = OFF_W3 + SZ_W3
OFF_SIN = OFF_COS + SZ_CS
BLOB = OFF_SIN + SZ_CS

_CACHE = {}


def _build(scale: float, reps: int = 1):
    import concourse.bacc as bacc
    import concourse.mybir as mybir
    import concourse.tile as tile

    f32 = mybir.dt.float32
    MM = mybir.dt.bfloat16
    EPS = float(np.finfo(np.float32).eps)

    nc = bacc.Bacc("TRN2", target_bir_lowering=False, debug=False,
                   num_devices=N_CORES)

    blob_d = nc.dram_tensor("blob", [BLOB], MM, kind="ExternalInput")
    y_d = nc.dram_tensor("y", [2 * 128, NT], MM, kind="ExternalOutput")

    Sq = mybir.ActivationFunctionType.Square
    Sqrt = mybir.ActivationFunctionType.Sqrt
    Exp = mybir.ActivationFunctionType.Exp
    Copy = mybir.ActivationFunctionType.Copy
    is_ge = mybir.AluOpType.is_ge
    bypass = mybir.AluOpType.bypass
    RG = [list(range(N_CORES))]

    def blob2d(off, p, f):
        return blob_d[off:off + p * f].rearrange("(p f) -> p f", f=f)

    with tile.TileContext(nc) as tc:
        with tc.tile_pool(name="dram", bufs=1, space="DRAM") as dram, \
             tc.tile_pool(name="res", bufs=1) as res:
            # residents: rotated q/k (m-chunks q0,q1,k0,k1), v in
            # [token, hd] layout, cos/sin, all-ones, o-proj weights
            qk_sb = res.tile([128, 4 * NT], MM, tag="qk")
            v_sb = res.tile([128, (NT // 128) * (HPC * HD)], MM, tag="v")
            cs_sb = res.tile([128, 2 * T], MM, tag="cs")
            w3_sb = res.tile([128, DC * 256], MM, tag="w3")
            ones_sb = res.tile([128, 128], MM, tag="ones")
            eps_sb = res.tile([128, 1], f32, tag="eps")
            nc.vector.memset(eps_sb[:], EPS)
            nc.vector.memset(ones_sb[:], 1.0)

            for _rep in range(reps):
                # Shared collective outputs allow a single writer inst, so
                # allocate fresh AG tiles per rep (reps>1 is diagnostics-only)
                agx_in = [dram.tile([256, 1024], MM, tag=f"agx_in{g}",
                                    name=f"agx_in{g}_r{_rep}") for g in range(4)]
                agx_out = [dram.tile([D, 1024], MM, tag=f"agx_out{g}",
                                     name=f"agx_out{g}_r{_rep}",
                                     addr_space="Shared") for g in range(4)]
                agy_in = [dram.tile([128, NT], MM, tag=f"agy_in{h}",
                                    name=f"agy_in{h}_r{_rep}") for h in range(HPC)]
                agy_out = [dram.tile([1024, NT], MM, tag=f"agy_out{h}",
                                     name=f"agy_out{h}_r{_rep}",
                                     addr_space="Shared") for h in range(HPC)]
                # x AllGather, chunked by 1024-token column groups so phase 1
                # can start after the first chunk lands
                for g in range(4):
                    for hf in range(2):
                        nc.sync.dma_start(
                            out=agx_in[g][hf * 128:(hf + 1) * 128, :],
                            in_=blob2d(OFF_X + g * 256 * 1024 + hf * 128 * 1024,
                                       128, 1024))
                    nc.gpsimd.collective_compute(
                        "AllGather", bypass, replica_groups=RG,
                        ins=[agx_in[g].opt()], outs=[agx_out[g].opt()])

                # cos/sin: C = [cosT; cosT], S = [-sinT; sinT]
                nc.sync.dma_start(out=cs_sb[0:64, 0:T],
                                  in_=blob2d(OFF_COS, 64, T))
                nc.sync.dma_start(out=cs_sb[64:128, 0:T],
                                  in_=blob2d(OFF_COS, 64, T))
                nc.sync.dma_start(out=cs_sb[64:128, T:2 * T],
                                  in_=blob2d(OFF_SIN, 64, T))
                nc.sync.dma_start(out=cs_sb[0:64, T:2 * T],
                                  in_=blob2d(OFF_SIN, 64, T))
                nc.scalar.activation(cs_sb[0:64, T:2 * T],
                                     cs_sb[0:64, T:2 * T], Copy, scale=-1.0)
                nc.sync.dma_start(out=w3_sb[:],
                                  in_=blob2d(OFF_W3, 128, DC * 256))

                # ---------------- Phase 1: QKV + RMS norm + rotary ----------------
                with tc.tile_pool(name="p1", bufs=1) as p1, \
                     tc.tile_pool(name="xs", bufs=3) as xs, \
                     tc.tile_pool(name="st", bufs=3) as st, \
                     tc.tile_pool(name="ps1", bufs=2, space="PSUM") as ps1:
                    wqk_sb = p1.tile([128, DC * 512], MM, tag="wqk")
                    wv_sb = p1.tile([128, DC * 256], MM, tag="wv")
                    nc.sync.dma_start(out=wv_sb[:],
                                      in_=blob2d(OFF_WV, 128, DC * 256))
                    nc.sync.dma_start(out=wqk_sb[:],
                                      in_=blob2d(OFF_WQK, 128, DC * 512))

                    for n in range(NTT):
                        xblk = xs.tile([128, DC * 512], MM, tag="xblk")
                        nh = (n % 2) * 512
                        for cg in range(4):
                            nc.sync.dma_start(
                                out=xblk[:, cg * 4 * 512:(cg + 1) * 4 * 512]
                                    .rearrange("p (c f) -> p c f", f=512),
                                in_=agx_out[n // 2][cg * 512:(cg + 1) * 512,
                                                    nh:nh + 512]
                                    .rearrange("(c p) f -> p c f", p=128))
                        # v projection: [token, hd] layout
                        for c4 in range(4):
                            vps = ps1.tile([128, HPC * HD], f32, tag="vps")
                            for dc in range(DC):
                                nc.tensor.matmul(
                                    vps[:],
                                    xblk[:, dc * 512 + c4 * 128: dc * 512 + (c4 + 1) * 128],
                                    wv_sb[:, dc * 256:(dc + 1) * 256],
                                    start=(dc == 0), stop=(dc == DC - 1))
                            tcg = n * 4 + c4
                            nc.vector.tensor_copy(v_sb[:, tcg * 256:(tcg + 1) * 256], vps[:])
                        # q/k projection + rms + rotary, m-chunks q0,q1,k0,k1
                        for m in range(4):
                            qps = ps1.tile([128, 512], f32, tag="qps")
                            for dc in range(DC):
                                nc.tensor.matmul(
                                    qps[:],
                                    wqk_sb[:, dc * 512 + m * 128: dc * 512 + (m + 1) * 128],
                                    xblk[:, dc * 512:(dc + 1) * 512],
                                    start=(dc == 0), stop=(dc == DC - 1))
                            sq = st.tile([128, 512], MM, tag="sq")
                            nc.scalar.activation(sq[:], qps[:], Sq)
                            ssq = ps1.tile([128, 512], f32, tag="ssq")
                            nc.tensor.matmul(ssq[:], ones_sb[:], sq[:], start=True, stop=True)
                            rms = st.tile([128, 512], f32, tag="rms")
                            nc.scalar.activation(rms[:], ssq[:], Sqrt, bias=eps_sb[:], scale=1.0 / HD)
                            r = st.tile([128, 512], f32, tag="r")
                            nc.vector.reciprocal(r[:], rms[:])
                            qn = st.tile([128, 512], MM, tag="qn")
                            nc.vector.tensor_mul(qn[:], qps[:], r[:])
                            # rotary: y = qn*C + swap(qn)*S  with S = [-sin; sin]
                            tsw = st.tile([128, 512], MM, tag="tsw")
                            tb = (n % 4) * 512
                            ctile = cs_sb[:, tb:tb + 512]
                            stile = cs_sb[:, T + tb:T + tb + 512]
                            nc.vector.tensor_mul(tsw[0:64, :], qn[64:128, :], stile[64:128, :])
                            nc.vector.tensor_mul(tsw[64:128, :], qn[0:64, :], stile[0:64, :])
                            dst = qk_sb[:, m * NT + n * 512: m * NT + (n + 1) * 512]
                            nc.vector.tensor_mul(dst, qn[:], ctile)
                            nc.vector.tensor_add(dst, dst, tsw[:])

                # ---------------- Phase 2: attention + per-head AllGather ----------------
                with tc.tile_pool(name="p2", bufs=6) as p2, \
                     tc.tile_pool(name="p2b", bufs=2) as p2b, \
                     tc.tile_pool(name="pss", bufs=3, space="PSUM") as pss, \
                     tc.tile_pool(name="psd", bufs=2, space="PSUM") as psd, \
                     tc.tile_pool(name="psy", bufs=2, space="PSUM") as psy:
                    for h in range(HPC):
                        qoff = h * NT
                        koff = (2 + h) * NT
                        for b in range(B):
                            for qj in range(4):
                                yps = psy.tile([128, 512], f32, tag="yps")
                                dps = psd.tile([128, 512], f32, tag="dps")
                                nkt = 4 * qj + 4
                                qbase = qoff + b * T + qj * 512
                                for kb in range(nkt):
                                    # diagonal blocks: only q-columns >= 128*m live
                                    lo = max(0, (kb - 4 * qj) * 128)
                                    sps = pss.tile([128, 512], f32, tag="sps")
                                    nc.tensor.matmul(
                                        sps[:, lo:],
                                        qk_sb[:, koff + b * T + kb * 128: koff + b * T + (kb + 1) * 128],
                                        qk_sb[:, qbase + lo: qbase + 512],
                                        start=True, stop=True)
                                    e = p2.tile([128, 512], MM, tag="e")
                                    nc.scalar.activation(e[:, lo:], sps[:, lo:], Exp, scale=scale)
                                    if kb >= 4 * qj:
                                        # causal: keep col j (>=lo) iff local
                                        # k-row p <= j - lo  (j - p - lo >= 0)
                                        nc.gpsimd.affine_select(
                                            out=e[:, lo:], in_=e[:, lo:],
                                            pattern=[[1, 512 - lo]],
                                            compare_op=is_ge, fill=0.0,
                                            base=0, channel_multiplier=-1)
                                    nc.tensor.matmul(dps[:, lo:], ones_sb[:], e[:, lo:],
                                                     start=(kb == 0), stop=(kb == nkt - 1))
                                    tcg = b * KT_PER_B + kb
                                    nc.tensor.matmul(
                                        yps[:, lo:],
                                        v_sb[:, tcg * 256 + h * 128: tcg * 256 + (h + 1) * 128],
                                        e[:, lo:],
                                        start=(kb == 0), stop=(kb == nkt - 1))
                                rcp = p2b.tile([128, 512], f32, tag="rcp")
                                nc.vector.reciprocal(rcp[:], dps[:])
                                yn = p2b.tile([128, 512], MM, tag="yn")
                                nc.vector.tensor_mul(yn[:], yps[:], rcp[:])
                                s = b * 4 + qj
                                nc.sync.dma_start(
                                    out=agy_in[h][:, s * 512:(s + 1) * 512],
                                    in_=yn[:])
                        # head h's AllGather overlaps head h+1's attention
                        nc.gpsimd.collective_compute(
                            "AllGather", bypass, replica_groups=RG,
                            ins=[agy_in[h].opt()], outs=[agy_out[h].opt()])

                # ---------------- Phase 3: column-parallel o-proj ----------------
                # ytile d-chunk dc: rows of agy_out[dc//8] block (dc%8);
                # w3 host layout is permuted to match. Two-pass accumulation:
                # dc 0-7 (head-0 data, available after AG#0) for 8 chains
                # first, so the tensor engine has work while AG#1 lands.
                with tc.tile_pool(name="yt", bufs=4) as ytp, \
                     tc.tile_pool(name="ob", bufs=4) as obp, \
                     tc.tile_pool(name="ps3", bufs=8, space="PSUM") as ps3:
                    for grp in range(2):
                        yts = []
                        for tl in range(4):
                            tt = grp * 4 + tl
                            ytile = ytp.tile([128, DC * 512], MM, tag="ytile")
                            for h in range(HPC):
                                nc.sync.dma_start(
                                    out=ytile[:, h * 8 * 512:(h + 1) * 8 * 512]
                                        .rearrange("p (c f) -> p c f", f=512),
                                    in_=agy_out[h][0:1024, tt * 512:(tt + 1) * 512]
                                        .rearrange("(c p) f -> p c f", p=128))
                            yts.append(ytile)
                        pst = [ps3.tile([128, 512], f32, tag="ops",
                                        name=f"ops{grp}_{i}")
                               for i in range(8)]
                        for half in range(2):
                            for tl in range(4):
                                for oc2 in range(2):
                                    ps = pst[tl * 2 + oc2]
                                    for dc in range(half * 8, half * 8 + 8):
                                        nc.tensor.matmul(
                                            ps[:],
                                            w3_sb[:, dc * 256 + oc2 * 128: dc * 256 + (oc2 + 1) * 128],
                                            yts[tl][:, dc * 512:(dc + 1) * 512],
                                            start=(dc == 0), stop=(dc == DC - 1))
                        for tl in range(4):
                            tt = grp * 4 + tl
                            for oc2 in range(2):
                                ob = obp.tile([128, 512], MM, tag="ob")
                                nc.scalar.activation(ob[:], pst[tl * 2 + oc2][:], Copy)
                                nc.sync.dma_start(
                                    out=y_d[oc2 * 128:(oc2 + 1) * 128,
                                            tt * 512:(tt + 1) * 512],
                                    in_=ob[:])

    nc.compile()
    return nc


def _prep_inputs(x, W, cos, sin):
    import concourse.mybir as mybir
    bf = mybir.dt.np(mybir.dt.bfloat16)

    xT = np.ascontiguousarray(x.reshape(NT, D).T).astype(bf)  # [D, NT]
    cosT = cos.T.astype(bf)  # [64, T]
    sinT = sin.T.astype(bf)
    W3T = W[3].T  # [d_in, d_out]
    # AllGather row order: AG#h stacks head (2c+h) of core c at block c
    blocks = [2 * c for c in range(N_CORES)] + [2 * c + 1 for c in range(N_CORES)]
    rows = np.concatenate([np.arange(b * 128, (b + 1) * 128) for b in blocks])
    W3p = W3T[rows]  # [d_in permuted, d_out]

    in_maps = []
    for c in range(N_CORES):
        r0 = c * HPC * HD
        wqk = np.concatenate([W[0][r0:r0 + 256], W[1][r0:r0 + 256]], 0).T  # [D, 512]
        wqk_sb = wqk.reshape(DC, 128, 512).transpose(1, 0, 2)
        wv = W[2][r0:r0 + 256].T  # [D, 256]
        wv_sb = wv.reshape(DC, 128, 256).transpose(1, 0, 2)
        w3_sb = W3p[:, r0:r0 + 256].reshape(DC, 128, 256).transpose(1, 0, 2)

        blob = np.empty(BLOB, bf)
        xs = xT[c * 256:(c + 1) * 256]
        for g in range(4):
            blob[OFF_X + g * 256 * 1024:OFF_X + (g + 1) * 256 * 1024] = \
                np.ascontiguousarray(xs[:, g * 1024:(g + 1) * 1024]).reshape(-1)
        blob[OFF_WQK:OFF_WQK + SZ_WQK] = wqk_sb.astype(bf).reshape(128, -1).reshape(-1)
        blob[OFF_WV:OFF_WV + SZ_WV] = wv_sb.astype(bf).reshape(128, -1).reshape(-1)
        blob[OFF_W3:OFF_W3 + SZ_W3] = w3_sb.astype(bf).reshape(128, -1).reshape(-1)
        blob[OFF_COS:OFF_COS + SZ_CS] = cosT.reshape(-1)
        blob[OFF_SIN:OFF_SIN + SZ_CS] = sinT.reshape(-1)
        in_maps.append({"blob": blob})
    return in_maps


def kernel(x, W, cos, sin, scale):
    from concourse.bass_utils import run_bass_kernel_spmd

    x = np.asarray(x, dtype=np.float32)
    W = np.asarray(W, dtype=np.float32)
    cos = np.asarray(cos, dtype=np.float32)
    sin = np.asarray(sin, dtype=np.float32)
    sc = float(np.asarray(scale))

    if sc not in _CACHE:
        _CACHE[sc] = _build(sc)
    nc = _CACHE[sc]

    in_maps = _prep_inputs(x, W, cos, sin)
    out = run_bass_kernel_spmd(nc, in_maps, core_ids=list(range(N_CORES)))
    yT = np.concatenate(
        [np.asarray(out.results[c]["y"], dtype=np.float32) for c in range(N_CORES)],
        axis=0)  # [D, NT]
    return np.ascontiguousarray(yT.T).reshape(B, T, D)
